# revision 1
# baseline (speedup 1.0000x reference)
"""DialogueEIN fused kernel for 8 TRN2 NeuronCores (data-parallel over batch).

Self-contained: hardcodes shapes for the nn_DialogueEIN problem
  x[64,256,512], T=256, H=512, NH=8 heads, E=7 emotion slots, window 5.

Strategy (per core, 8 batches, processed as 4 batch-PAIRS):
  - All activations live in "transposed" space [H, T] so attention scores are
    computed directly as S_T[k, j] (keys on partitions, queries on free dim):
    qT/kT come straight out of the projection matmuls; softmax needs no
    transposes anywhere.  Two batches share each tile on the free dim
    ([128, 512]) so projection/LN/exp instruction counts halve.
  - Softmax without max-subtraction: scores are O(1) and the additive mask
    bias is -50 instead of -1e4 (identical through softmax: fully-masked rows
    reduce to the reference's plain softmax; partially-masked rows leave
    masked weights at ~e^-48 relative -- below fp32 noise).
  - The PV matmul's lhsT is a contiguous [V_h | ones64] 128-column group, so
    PSUM rows 64:128 hold 64 broadcast copies of the softmax denominator row;
    normalization is a 64-lane reciprocal + the eviction multiply.
  - Mask biases for global/intra/inter are rank<=3 outer products accumulated
    into the score PSUM by tiny extra matmuls (host ships factor vectors).
    The local sliding-window branch uses a multiplicative post-exp mask
    built per pair from a constant band matrix (4 rank-1 matmuls + 2 DVE ops).
  - Host folds: b_Wo[i] @ W1_i (kills the concat+W1 matmul), ln2 gamma/beta
    into W2, t_bv/b_bv into downstream biases, 1/sqrt(dh) into Wq.
  - LayerNorm over the partition axis: ones-column matmuls for mean/E[x^2],
    PE rank-1 broadcast of rstd / (-mu*rstd) rows, per-partition gamma/beta.
  - All big matmuls run as float32r (full-rate fp32 mode on the PE).
"""

import numpy as np

import concourse.bass as bass
import concourse.mybir as mybir
import concourse.tile as tile
from concourse import bacc
from concourse.bass_utils import run_bass_kernel_spmd

F32 = mybir.dt.float32
F32R = mybir.dt.float32r
ALU = mybir.AluOpType
ACTF = mybir.ActivationFunctionType

B, T, H, NH, E = 64, 256, 512, 8, 7
DH = H // NH
NCORES = 8
BL = B // NCORES          # 8 batches per core
PR = BL // 2              # 4 batch-pairs per core
WD = 2 * T                # 512: paired free width
VW = NH * 128             # 1024: V_aug width ([V_h | ones64] per head)
NEG = -50.0
KT = H // 128             # 4
MT = T // 128             # 2
EPS = 1e-12

_CACHE = {}


def _build(apply_g1b1, apply_c2):
    nc = bacc.Bacc("TRN2", target_bir_lowering=False, debug=False,
                   enable_asserts=False)

    def din(name, shape, dt=F32R):
        return nc.dram_tensor(name, list(shape), dt, kind="ExternalInput").ap()

    xT = din("xT", (PR, H, WD))
    smalls = din("smalls", (BL, 3, 9, T))
    bandd = din("bandd", (T, WD), mybir.dt.bfloat16)
    kTemo = din("kTemo", (H, E))
    vemoaug = din("vemoaug", (E, VW))
    tWq = din("tWq", (H, H))
    tWo = din("tWo", (H, H))
    bWq = din("bWq", (4, H, H))
    bWk = din("bWk", (4, H, H))
    bWv = din("bWv", (4, H, H))
    What = din("What", (4, H, H))
    W2p = din("W2p", (H, H))
    onesd = din("onesd", (T,))
    tbq = din("tbq", (H,), F32)
    tbo = din("tbo", (H,), F32)
    g1 = din("g1", (H,), F32)
    b1v = din("b1v", (H,), F32)
    bbq = din("bbq", (4, H), F32)
    bbk = din("bbk", (4, H), F32)
    bhat = din("bhat", (H,), F32)
    c2row = din("c2row", (H,), F32)
    out = nc.dram_tensor("out", [BL, T, H], F32, kind="ExternalOutput").ap()

    with tile.TileContext(nc) as tc:
        cst = tc.alloc_tile_pool(name="cst", bufs=1)
        per = tc.alloc_tile_pool(name="per", bufs=1)
        wts = tc.alloc_tile_pool(name="wts", bufs=1)
        act = tc.alloc_tile_pool(name="act", bufs=1)
        pmm = tc.alloc_tile_pool(name="pmm", bufs=3, space="PSUM")
        psc = tc.alloc_tile_pool(name="psc", bufs=3, space="PSUM")
        pcx = tc.alloc_tile_pool(name="pcx", bufs=2, space="PSUM")

        # ---- constants ----
        ones128 = cst.tile([128, 1], F32R, name="ones128")
        nc.sync.dma_start(out=ones128, in_=onesd[0:128])
        onr32 = cst.tile([1, 128], F32R, name="onr32")
        nc.sync.dma_start(out=onr32, in_=onesd[0:128])
        eps_t = cst.tile([1, 1], F32, name="eps_t")
        nc.vector.memset(eps_t, EPS)
        kTe = []
        for k in range(KT):
            t = cst.tile([128, E], F32R, name=f"kTemo{k}")
            nc.sync.dma_start(out=t, in_=kTemo[k * 128:(k + 1) * 128, :])
            kTe.append(t)
        vea = cst.tile([E, VW], F32R, name="vemoaug")
        nc.sync.dma_start(out=vea, in_=vemoaug)
        bandt = []
        for m in range(MT):
            t = cst.tile([128, WD], mybir.dt.bfloat16, name=f"band{m}")
            nc.sync.dma_start(out=t, in_=bandd[m * 128:(m + 1) * 128, :])
            bandt.append(t)

        def vec_tiles(v, nm):
            ts = []
            for k in range(KT):
                t = cst.tile([128, 1], F32, name=f"{nm}{k}")
                nc.sync.dma_start(out=t, in_=v[k * 128:(k + 1) * 128])
                ts.append(t)
            return ts

        tbq_t = vec_tiles(tbq, "tbq")
        tbo_t = vec_tiles(tbo, "tbo")
        bhat_t = vec_tiles(bhat, "bhat")
        g1_t = vec_tiles(g1, "g1") if apply_g1b1 else None
        b1_t = vec_tiles(b1v, "b1v") if apply_g1b1 else None
        bbq_t = [vec_tiles(bbq[i], f"bbq{i}") for i in range(4)]
        bbk_t = [vec_tiles(bbk[i], f"bbk{i}") for i in range(4)]
        c2n = None
        if apply_c2:
            c2n = cst.tile([128, H], F32, name="c2n")
            nc.sync.dma_start(
                out=c2n, in_=bass.AP(tensor=c2row.tensor, offset=c2row.offset,
                                     ap=[[0, 128], [1, H]]))

        # persistent per-pair state
        htT = [[per.tile([128, WD], F32R, name=f"htT_{p}_{k}")
                for k in range(KT)] for p in range(PR)]
        h2sb = [[per.tile([128, WD], F32R, name=f"h2sb_{p}_{k}")
                 for k in range(KT)] for p in range(PR)]

        def proj_T(wtiles, rhs_tiles, bias_tiles, tag):
            """[H, WD] = W.T @ rhs(pair), +bias per-partition (ACT evict)."""
            res = []
            for mo in range(KT):
                ps = pmm.tile([128, WD], F32, tag="mm", bufs=3, name="psp")
                for ki in range(KT):
                    nc.tensor.matmul(
                        ps, wtiles[ki][:, mo * 128:(mo + 1) * 128],
                        rhs_tiles[ki], start=(ki == 0), stop=(ki == KT - 1))
                s = act.tile([128, WD], F32R, tag=tag, bufs=4, name="proj")
                nc.scalar.activation(s, ps, ACTF.Identity, bias=bias_tiles[mo])
                res.append(s)
            return res

        def softmax_pv2(h, e_tiles, va2, ctxTn):
            """PV for BOTH batch halves of one head into one [128, WD] psum;
            lhsT = [V_h | ones64] contiguous group so psum rows 64:128 hold
            the denominator rows; one reciprocal + one multiply-evict."""
            ps = pcx.tile([128, WD], F32, tag="ctx", bufs=2, name="ctxps")
            nkt = len(e_tiles)
            for bh in range(2):
                hsl = slice(bh * T, (bh + 1) * T)
                for kt in range(nkt):
                    nc.tensor.matmul(ps[:, hsl],
                                     va2[bh][kt][:, h * 128:(h + 1) * 128],
                                     e_tiles[kt][:, hsl], start=(kt == 0),
                                     stop=(kt == nkt - 1))
            recD = act.tile([64, WD], F32, tag="recD", bufs=2, name="recD")
            nc.vector.reciprocal(out=recD, in_=ps[64:128, :])
            pi = (h % 2) * 64
            nc.vector.tensor_tensor(out=ctxTn[h // 2][pi:pi + 64, :],
                                    in0=ps[0:64, :], in1=recD, op=ALU.mult)

        def bcast_row(row_ap):
            t = pcx.tile([128, WD], F32, tag="ctx", bufs=2, name="bcast")
            nc.tensor.matmul(t, onr32, row_ap, start=True, stop=True)
            return t

        def layer_norm_T(s_tiles, gb, dests=None):
            """LN over the partition (H) axis of transposed [H, WD] pair."""
            psmu = psc.tile([128, WD], F32, tag="sc", bufs=3, name="psmu")
            for k in range(KT):
                nc.tensor.matmul(psmu[0:1, :], ones128, s_tiles[k],
                                 start=(k == 0), stop=(k == KT - 1))
            pss2 = psc.tile([128, WD], F32, tag="sc", bufs=3, name="pss2")
            for k in range(KT):
                sq = act.tile([128, WD], F32R, tag="sq", bufs=2, name="sq")
                nc.scalar.activation(sq, s_tiles[k], ACTF.Square)
                nc.tensor.matmul(pss2[0:1, :], ones128, sq,
                                 start=(k == 0), stop=(k == KT - 1))

            def stat():
                return act.tile([1, WD], F32R, tag="lnstat", bufs=5,
                                name="lnstat")
            mu, ex2, var, rstd, nm = (stat() for _ in range(5))
            nc.scalar.activation(mu, psmu[0:1, :], ACTF.Copy, scale=1.0 / H)
            nc.scalar.activation(ex2, pss2[0:1, :], ACTF.Copy, scale=1.0 / H)
            nc.vector.scalar_tensor_tensor(var, mu, -1.0, mu,
                                           op0=ALU.mult, op1=ALU.mult)
            nc.vector.tensor_tensor(out=var, in0=ex2, in1=var, op=ALU.add)
            nc.scalar.activation(var, var, ACTF.Sqrt, bias=eps_t)
            with nc.allow_low_precision("f32r rows feed broadcast matmuls"):
                nc.vector.reciprocal(rstd, var)
            nc.vector.scalar_tensor_tensor(nm, mu, -1.0, rstd,
                                           op0=ALU.mult, op1=ALU.mult)
            RS = bcast_row(rstd)
            NM = bcast_row(nm)
            res = []
            for k in range(KT):
                o = (dests[k] if dests is not None else
                     act.tile([128, WD], F32R, tag="lno", bufs=4, name="lno"))
                nc.vector.tensor_tensor(out=o, in0=s_tiles[k], in1=RS,
                                        op=ALU.mult)
                nc.vector.tensor_tensor(out=o, in0=o, in1=NM, op=ALU.add)
                if gb is not None:
                    nc.vector.tensor_scalar(o, o, gb[0][k], gb[1][k],
                                            op0=ALU.mult, op1=ALU.add)
                res.append(o)
            return res

        # ---------------- Stage T: tendency attention + LN1 ----------------
        wq_t = [wts.tile([128, H], F32R, tag="w", bufs=16, name="twq")
                for _ in range(KT)]
        wo_t = [wts.tile([128, H], F32R, tag="w", bufs=16, name="two")
                for _ in range(KT)]
        for k in range(KT):
            nc.sync.dma_start(out=wq_t[k], in_=tWq[k * 128:(k + 1) * 128, :])
            nc.sync.dma_start(out=wo_t[k], in_=tWo[k * 128:(k + 1) * 128, :])

        for p in range(PR):
            xTb = []
            for k in range(KT):
                t = act.tile([128, WD], F32R, tag="xT", bufs=4, name="xTt")
                nc.sync.dma_start(out=t, in_=xT[p, k * 128:(k + 1) * 128, :])
                xTb.append(t)
            qT = proj_T(wq_t, xTb, tbq_t, "qT")
            ctxTn = [act.tile([128, WD], F32R, tag="ctxTn", bufs=4,
                              name="ctxTn") for _ in range(KT)]
            for h in range(NH):
                pi = (h % 2) * 64
                sps = psc.tile([128, WD], F32, tag="sc", bufs=3, name="scte")
                nc.tensor.matmul(sps[0:E, :], kTe[h // 2][pi:pi + 64, :],
                                 qT[h // 2][pi:pi + 64, :],
                                 start=True, stop=True)
                et = act.tile([E, WD], F32R, tag="et", bufs=3, name="ett")
                nc.scalar.activation(et, sps[0:E, :], ACTF.Exp)
                softmax_pv2(h, [et[0:E, :]], [[vea], [vea]], ctxTn)
            s1 = []
            for mo in range(KT):
                ps = pmm.tile([128, WD], F32, tag="mm", bufs=3, name="psh")
                for ki in range(KT):
                    nc.tensor.matmul(ps, wo_t[ki][:, mo * 128:(mo + 1) * 128],
                                     ctxTn[ki], start=(ki == 0),
                                     stop=(ki == KT - 1))
                s = act.tile([128, WD], F32R, tag="s1", bufs=4, name="s1")
                nc.vector.scalar_tensor_tensor(s, ps, tbo_t[mo], xTb[mo],
                                               op0=ALU.add, op1=ALU.add)
                s1.append(s)
            layer_norm_T(s1, (g1_t, b1_t) if apply_g1b1 else None,
                         dests=htT[p])

        # ---------------- Branch stages ----------------
        for i in range(4):
            wq_b = [wts.tile([128, H], F32R, tag="w", bufs=16, name="bwq")
                    for _ in range(KT)]
            wk_b = [wts.tile([128, H], F32R, tag="w", bufs=16, name="bwk")
                    for _ in range(KT)]
            wv_b = [wts.tile([128, H], F32R, tag="w", bufs=16, name="bwv")
                    for _ in range(KT)]
            wh_b = [wts.tile([128, H], F32R, tag="w", bufs=16, name="bwh")
                    for _ in range(KT)]
            for k in range(KT):
                sl = slice(k * 128, (k + 1) * 128)
                nc.sync.dma_start(out=wq_b[k], in_=bWq[i, sl, :])
                nc.sync.dma_start(out=wk_b[k], in_=bWk[i, sl, :])
                nc.sync.dma_start(out=wv_b[k], in_=bWv[i, sl, :])
                nc.sync.dma_start(out=wh_b[k], in_=What[i, sl, :])
            gsl = {0: slice(0, 2), 1: slice(5, 9),
                   2: slice(1, 3), 3: slice(3, 5)}[i]
            ng = gsl.stop - gsl.start
            for p in range(PR):
                sms = []
                for bh in range(2):
                    sm = act.tile([3, 4, T], F32R, tag="sm", bufs=2, name="sm",
                                  padded_shape=None)
                    sm = sm[:, 0:ng, :]
                    nc.sync.dma_start(out=sm,
                                      in_=smalls[2 * p + bh][:, gsl, :])
                    sms.append(sm)
                ml = None
                if i == 1:  # local: multiplicative mask band*outer(col,row)+B
                    ml = []
                    for m in range(MT):
                        msl = slice(m * 128, (m + 1) * 128)
                        psA = psc.tile([128, WD], F32, tag="sc", bufs=3,
                                       name="psA")
                        psB = psc.tile([128, WD], F32, tag="sc", bufs=3,
                                       name="psB")
                        for bh in range(2):
                            hsl = slice(bh * T, (bh + 1) * T)
                            nc.tensor.matmul(psA[:, hsl], sms[bh][0:1, 0, msl],
                                             sms[bh][0:1, 1, :],
                                             start=True, stop=True)
                            nc.tensor.matmul(psB[:, hsl], sms[bh][0:1, 3, msl],
                                             sms[bh][0:1, 2, :],
                                             start=True, stop=True)
                        mt_ = act.tile([128, WD], F32, tag="ml", bufs=2,
                                       name="ml")
                        nc.vector.tensor_tensor(out=mt_, in0=psA,
                                                in1=bandt[m], op=ALU.mult)
                        nc.vector.tensor_tensor(out=mt_, in0=mt_, in1=psB,
                                                op=ALU.add)
                        ml.append(mt_)
                qT = proj_T(wq_b, htT[p], bbq_t[i], "qT")
                kT = proj_T(wk_b, htT[p], bbk_t[i], "kT")
                va = [[None] * MT for _ in range(2)]
                for bh in range(2):
                    for mo in range(MT):
                        ps = pmm.tile([128, H], F32, tag="mm", bufs=3,
                                      name="psv")
                        for ki in range(KT):
                            off = bh * T + mo * 128
                            nc.tensor.matmul(
                                ps, htT[p][ki][:, off:off + 128],
                                wv_b[ki], start=(ki == 0), stop=(ki == KT - 1))
                        vt = act.tile([128, VW], F32R, tag="va", bufs=4,
                                      name="va")
                        vt3 = vt.rearrange("p (h d) -> p h d", h=NH)
                        ps3 = ps.rearrange("p (h d) -> p h d", h=NH)
                        nc.scalar.activation(vt3[:, :, 0:64], ps3, ACTF.Copy)
                        nc.vector.memset(vt3[:, :, 64:128].bitcast(
                            mybir.dt.uint32), 0x3F800000)
                        va[bh][mo] = vt
                ctxTn = [act.tile([128, WD], F32R, tag="ctxTn", bufs=4,
                                  name="ctxTn") for _ in range(KT)]
                for h in range(NH):
                    pi = (h % 2) * 64
                    ets = []
                    for m in range(MT):
                        msl = slice(m * 128, (m + 1) * 128)
                        sps = psc.tile([128, WD], F32, tag="sc", bufs=3,
                                       name="scb")
                        for bh in range(2):
                            hsl = slice(bh * T, (bh + 1) * T)
                            nc.tensor.matmul(
                                sps[:, hsl],
                                kT[h // 2][pi:pi + 64, bh * T + m * 128:
                                           bh * T + m * 128 + 128],
                                qT[h // 2][pi:pi + 64, hsl],
                                start=True, stop=(i == 1))
                            if i == 0:
                                nc.tensor.matmul(sps[:, hsl],
                                                 sms[bh][0:1, 0, msl],
                                                 sms[bh][0:1, 1, :],
                                                 start=False, stop=True)
                            elif i in (2, 3):
                                nc.tensor.matmul(sps[:, hsl],
                                                 sms[bh][:, 1, msl],
                                                 sms[bh][:, 0, :],
                                                 start=False, stop=True)
                        et = act.tile([128, WD], F32R, tag="et", bufs=3,
                                      name="etb")
                        nc.scalar.activation(et, sps, ACTF.Exp)
                        if i == 1:
                            nc.vector.tensor_tensor(out=et, in0=et, in1=ml[m],
                                                    op=ALU.mult)
                        ets.append(et)
                    softmax_pv2(h, ets, va, ctxTn)
                for mo in range(KT):
                    ps = pmm.tile([128, WD], F32, tag="mm", bufs=3,
                                  name="psh2")
                    for ki in range(KT):
                        nc.tensor.matmul(
                            ps, wh_b[ki][:, mo * 128:(mo + 1) * 128],
                            ctxTn[ki], start=(ki == 0), stop=(ki == KT - 1))
                    if i == 0:
                        nc.scalar.copy(out=h2sb[p][mo], in_=ps)
                    elif i < 3:
                        nc.vector.tensor_tensor(out=h2sb[p][mo],
                                                in0=h2sb[p][mo], in1=ps,
                                                op=ALU.add)
                    else:
                        nc.vector.scalar_tensor_tensor(
                            h2sb[p][mo], ps, bhat_t[mo], h2sb[p][mo],
                            op0=ALU.add, op1=ALU.add)

        # ---------------- Final: residual + LN2 + W2 ----------------
        w2_t = [wts.tile([128, H], F32R, tag="w", bufs=16, name="w2p")
                for _ in range(KT)]
        for k in range(KT):
            nc.sync.dma_start(out=w2_t[k], in_=W2p[k * 128:(k + 1) * 128, :])
        for p in range(PR):
            for k in range(KT):
                nc.vector.tensor_tensor(out=h2sb[p][k], in0=h2sb[p][k],
                                        in1=htT[p][k], op=ALU.add)
            n2 = layer_norm_T(h2sb[p], None)
            for bh in range(2):
                for mo in range(MT):
                    ps = pmm.tile([128, H], F32, tag="mm", bufs=3, name="pso")
                    for ki in range(KT):
                        off = bh * T + mo * 128
                        nc.tensor.matmul(ps, n2[ki][:, off:off + 128],
                                         w2_t[ki], start=(ki == 0),
                                         stop=(ki == KT - 1))
                    osb = act.tile([128, H], F32, tag="osb", bufs=1,
                                   name="osb")
                    if apply_c2:
                        nc.vector.tensor_tensor(out=osb, in0=ps, in1=c2n,
                                                op=ALU.add)
                    else:
                        nc.scalar.copy(out=osb, in_=ps)
                    nc.sync.dma_start(
                        out=out[2 * p + bh, mo * 128:(mo + 1) * 128, :],
                        in_=osb)
        pcx.release()
        psc.release()
        pmm.release()
        act.release()
        wts.release()
        per.release()
        cst.release()

    nc.compile()
    return nc


def _host_prep(inputs):
    f32 = np.float32
    g = {}
    x = np.asarray(inputs["x"], f32)
    lengths = np.asarray(inputs["lengths"])
    speakers = np.asarray(inputs["speakers"])
    emo = np.asarray(inputs["emo_table"], f32)

    xTa = np.ascontiguousarray(x.transpose(0, 2, 1))  # [B, H, T]
    xTp = np.ascontiguousarray(
        xTa.reshape(B // 2, 2, H, T).transpose(0, 2, 1, 3).reshape(
            B // 2, H, WD))
    j = np.arange(T)
    row = (j[None, :] < lengths[:, None]).astype(f32)
    col = row
    sp = speakers.astype(f32)
    u1 = row * sp
    u2 = row * (1.0 - sp)
    ones = np.ones_like(row)
    z = np.zeros_like(row)
    sm = np.zeros((B, 3, 9, T), f32)
    sm[:, 0, 0] = NEG * (1.0 - col)                               # 0: FR glob
    sm[:, 0, 1], sm[:, 1, 1], sm[:, 2, 1] = ones, u1, u2          # 1: FL
    sm[:, 0, 2], sm[:, 1, 2], sm[:, 2, 2] = (NEG * ones, -NEG * sp,
                                             -NEG * (1.0 - sp))   # 2: FRintra
    sm[:, 0, 3], sm[:, 1, 3], sm[:, 2, 3] = ones, u1, u2          # 3: FL dup
    sm[:, 0, 4], sm[:, 1, 4], sm[:, 2, 4] = (NEG * ones,
                                             -NEG * (1.0 - sp) * col,
                                             -NEG * sp * col)     # 4: FRinter
    sm[:, 0, 5] = col                                             # 5
    sm[:, 0, 6] = row                                             # 6
    sm[:, 0, 7] = 1.0 - row                                       # 7
    sm[:, 0, 8] = ones[0]                                         # 8

    import ml_dtypes
    band = (np.abs(j[:, None] - j[None, :]) <= 2)
    g["bandd"] = np.concatenate([band, band],
                                axis=1).astype(ml_dtypes.bfloat16)
    kemo = (emo @ np.asarray(inputs["t_Wk"], f32)
            + np.asarray(inputs["t_bk"], f32))
    g["kTemo"] = np.ascontiguousarray(kemo.T)
    vemo = (emo @ np.asarray(inputs["t_Wv"], f32)
            + np.asarray(inputs["t_bv"], f32))
    vaug = np.ones((E, VW), f32)
    vaug3 = vaug.reshape(E, NH, 128)
    vaug3[:, :, 0:64] = vemo.reshape(E, NH, 64)
    g["vemoaug"] = vaug
    g["tWq"] = np.asarray(inputs["t_Wq"], f32) / np.sqrt(DH).astype(f32)
    g["tWo"] = np.asarray(inputs["t_Wo"], f32)
    g["bWq"] = np.asarray(inputs["b_Wq"], f32) / np.sqrt(DH).astype(f32)
    g["bWk"] = np.asarray(inputs["b_Wk"], f32)
    g["bWv"] = np.asarray(inputs["b_Wv"], f32)
    W1 = np.asarray(inputs["W1"], np.float64)
    bWo = np.asarray(inputs["b_Wo"], np.float64)
    g["What"] = np.stack(
        [(bWo[i] @ W1[i * H:(i + 1) * H]).astype(f32) for i in range(4)])
    ln2g = np.asarray(inputs["ln2_g"], np.float64)
    g["W2p"] = (ln2g[:, None]
                * np.asarray(inputs["W2"], np.float64)).astype(f32)
    g["onesd"] = np.ones(T, f32)
    g["tbq"] = np.asarray(inputs["t_bq"], f32) / np.sqrt(DH).astype(f32)
    g["tbo"] = (np.asarray(inputs["t_bo"], np.float64)
                + np.asarray(inputs["t_bv"], np.float64)
                @ np.asarray(inputs["t_Wo"], np.float64)).astype(f32)
    g["g1"] = np.asarray(inputs["t_ln_g"], f32)
    g["b1v"] = np.asarray(inputs["t_ln_b"], f32)
    g["bbq"] = np.asarray(inputs["b_bq"], f32) / np.sqrt(DH).astype(f32)
    g["bbk"] = np.asarray(inputs["b_bk"], f32)
    bhat = np.asarray(inputs["b1"], np.float64).copy()
    for i in range(4):
        eff = (np.asarray(inputs["b_bo"][i], np.float64)
               + np.asarray(inputs["b_bv"][i], np.float64) @ bWo[i])
        bhat += eff @ W1[i * H:(i + 1) * H]
    g["bhat"] = bhat.astype(f32)
    g["c2row"] = (np.asarray(inputs["ln2_b"], np.float64)
                  @ np.asarray(inputs["W2"], np.float64)).astype(f32)

    apply_g1b1 = not (np.all(inputs["t_ln_g"] == 1.0)
                      and np.all(inputs["t_ln_b"] == 0.0))
    apply_c2 = bool(np.any(g["c2row"] != 0.0))

    in_maps = []
    for c in range(NCORES):
        m = dict(g)
        m["xT"] = np.ascontiguousarray(xTp[c * PR:(c + 1) * PR])
        m["smalls"] = np.ascontiguousarray(sm[c * BL:(c + 1) * BL])
        in_maps.append(m)
    return in_maps, apply_g1b1, apply_c2


def kernel(**inputs):
    in_maps, apply_g1b1, apply_c2 = _host_prep(inputs)
    key = (apply_g1b1, apply_c2)
    if key not in _CACHE:
        _CACHE[key] = _build(*key)
    nc = _CACHE[key]
    res = run_bass_kernel_spmd(nc, in_maps, core_ids=list(range(NCORES)),
                               trace=False)
    outs = [res.results[c]["out"] for c in range(NCORES)]
    return np.concatenate(outs, axis=0)



# revision 25
# speedup vs baseline: 1.0559x; 1.0559x over previous
"""DialogueEIN fused kernel for 8 TRN2 NeuronCores (data-parallel over batch).

Self-contained: hardcodes shapes for the nn_DialogueEIN problem
  x[64,256,512], T=256, H=512, NH=8 heads, E=7 emotion slots, window 5.

Strategy (per core, 8 batches, processed as 4 batch-PAIRS):
  - All activations live in "transposed" space [H, T] so attention scores are
    computed directly as S_T[k, j] (keys on partitions, queries on free dim):
    qT/kT come straight out of the projection matmuls; softmax needs no
    transposes anywhere.  Two batches share each tile on the free dim
    ([128, 512]) so projection/LN/exp instruction counts halve.
  - Softmax without max-subtraction: scores are O(1) and the additive mask
    bias is -50 instead of -1e4 (identical through softmax: fully-masked rows
    reduce to the reference's plain softmax; partially-masked rows leave
    masked weights at ~e^-48 relative -- below fp32 noise).
  - The PV matmul's lhsT is a contiguous [V_h | ones64] 128-column group, so
    PSUM rows 64:128 hold 64 broadcast copies of the softmax denominator row;
    normalization is a 64-lane reciprocal + the eviction multiply.
  - Mask biases for global/intra/inter are rank<=3 outer products accumulated
    into the score PSUM by tiny extra matmuls (host ships factor vectors).
    The local sliding-window branch uses a multiplicative post-exp mask
    built per pair from a constant band matrix (4 rank-1 matmuls + 2 DVE ops).
  - Host folds: b_Wo[i] @ W1_i (kills the concat+W1 matmul), ln2 gamma/beta
    into W2, t_bv/b_bv into downstream biases, 1/sqrt(dh) into Wq.
  - LayerNorm over the partition axis: ones-column matmuls for mean/E[x^2],
    PE rank-1 broadcast of rstd / (-mu*rstd) rows, per-partition gamma/beta.
  - All big matmuls run as float32r (full-rate fp32 mode on the PE).
"""

import numpy as np

import concourse.bass as bass
import concourse.mybir as mybir
import concourse.tile as tile
from concourse import bacc
from concourse.bass_utils import run_bass_kernel_spmd

F32 = mybir.dt.float32
F32R = mybir.dt.float32r
BF16 = mybir.dt.bfloat16
ALU = mybir.AluOpType
ACTF = mybir.ActivationFunctionType

B, T, H, NH, E = 64, 256, 512, 8, 7
DH = H // NH
NCORES = 8
BL = B // NCORES          # 8 batches per core
PR = BL // 2              # 4 batch-pairs per core
WD = 2 * T                # 512: paired free width
VW = NH * 128             # 1024: V_aug width ([V_h | ones64] per head)
NEG = -50.0
KT = H // 128             # 4
MT = T // 128             # 2
EPS = 1e-12

_CACHE = {}


def _build(apply_g1b1, apply_c2):
    nc = bacc.Bacc("TRN2", target_bir_lowering=False, debug=False,
                   enable_asserts=False)

    def din(name, shape, dt=F32R):
        return nc.dram_tensor(name, list(shape), dt, kind="ExternalInput").ap()

    xT = din("xT", (PR, H, WD))
    smalls = din("smalls", (PR, 3, 9, WD))
    bandd = din("bandd", (T, WD), mybir.dt.bfloat16)
    kTemo = din("kTemo", (H, E), BF16)
    vemoaug = din("vemoaug", (E, VW))
    tWq = din("tWq", (H, H))
    tWo = din("tWo", (H, H), BF16)
    bWq = din("bWq", (4, H, H), BF16)
    bWk = din("bWk", (4, H, H), BF16)
    bWv = din("bWv", (4, H, H), BF16)
    What = din("What", (4, H, H), BF16)
    W2p = din("W2p", (H, H))
    onesd = din("onesd", (T,))
    # packed per-partition vectors: [H, 16] cols:
    # 0 tbq, 1 tbo, 2 bhat, 3 g1, 4 b1v, 5:9 bbq[0..3], 9:13 bbk[0..3]
    vecs = din("vecs", (H, 16), F32)
    c2row = din("c2row", (H,), F32)
    out = nc.dram_tensor("out", [BL, T, H], F32, kind="ExternalOutput").ap()

    with tile.TileContext(nc) as tc:
        cst = tc.alloc_tile_pool(name="cst", bufs=1)
        per = tc.alloc_tile_pool(name="per", bufs=1)
        wts = tc.alloc_tile_pool(name="wts", bufs=1)
        act = tc.alloc_tile_pool(name="act", bufs=1)
        pmm = tc.alloc_tile_pool(name="pmm", bufs=3, space="PSUM")
        psc = tc.alloc_tile_pool(name="psc", bufs=3, space="PSUM")
        pcx = tc.alloc_tile_pool(name="pcx", bufs=2, space="PSUM")

        # ---- startup-critical loads first: tendency weights + pair-0 xT
        # interleaved so the first projection matmuls can begin ASAP.
        wq_t = [wts.tile([128, H], F32R, tag="w", bufs=8, name="twq")
                for _ in range(KT)]
        wo_t = [wts.tile([128, H], BF16, tag="wh", bufs=32, name="two")
                for _ in range(KT)]
        xTb0 = [act.tile([128, WD], F32R, tag="xT", bufs=4, name="xTt")
                for _ in range(KT)]
        for k in range(KT):
            nc.sync.dma_start(out=wq_t[k], in_=tWq[k * 128:(k + 1) * 128, :])
            nc.sync.dma_start(out=xTb0[k],
                              in_=xT[0, k * 128:(k + 1) * 128, :])
        vec_t = []
        for k in range(KT):
            t = cst.tile([128, 16], F32, name=f"vecs{k}")
            nc.sync.dma_start(out=t, in_=vecs[k * 128:(k + 1) * 128, :])
            vec_t.append(t)
        ones128 = cst.tile([128, 1], F32R, name="ones128")
        nc.sync.dma_start(out=ones128, in_=onesd[0:128])
        onr32 = cst.tile([1, 128], F32R, name="onr32")
        nc.sync.dma_start(out=onr32, in_=onesd[0:128])
        eps_t = cst.tile([1, 1], F32, name="eps_t")
        nc.vector.memset(eps_t, EPS)
        kTe = []
        for k in range(KT):
            t = cst.tile([128, E], BF16, name=f"kTemo{k}")
            nc.sync.dma_start(out=t, in_=kTemo[k * 128:(k + 1) * 128, :])
            kTe.append(t)
        vea = cst.tile([E, VW], F32R, name="vemoaug")
        nc.sync.dma_start(out=vea, in_=vemoaug)
        for k in range(KT):
            nc.sync.dma_start(out=wo_t[k], in_=tWo[k * 128:(k + 1) * 128, :])
        bandt = []
        for m in range(MT):
            t = cst.tile([128, WD], mybir.dt.bfloat16, name=f"band{m}")
            nc.sync.dma_start(out=t, in_=bandd[m * 128:(m + 1) * 128, :])
            bandt.append(t)

        def vcol(j):
            return [vec_t[k][:, j:j + 1] for k in range(KT)]

        tbq_t = vcol(0)
        tbo_t = vcol(1)
        bhat_t = vcol(2)
        g1_t = vcol(3) if apply_g1b1 else None
        b1_t = vcol(4) if apply_g1b1 else None
        bbq_t = [vcol(5 + i) for i in range(4)]
        bbk_t = [vcol(9 + i) for i in range(4)]
        c2n = None
        if apply_c2:
            c2n = cst.tile([128, H], F32, name="c2n")
            nc.sync.dma_start(
                out=c2n, in_=bass.AP(tensor=c2row.tensor, offset=c2row.offset,
                                     ap=[[0, 128], [1, H]]))

        # persistent per-pair state (htT bf16: feeds bf16 matmuls)
        htT = [[per.tile([128, WD], BF16, name=f"htT_{p}_{k}")
                for k in range(KT)] for p in range(PR)]
        h2sb = [[per.tile([128, WD], F32R, name=f"h2sb_{p}_{k}")
                 for k in range(KT)] for p in range(PR)]

        def proj_T(wtiles, rhs_tiles, bias_tiles, tag):
            """[H, WD] = W.T @ rhs(pair), +bias per-partition (ACT evict)."""
            res = []
            for mo in range(KT):
                ps = pmm.tile([128, WD], F32, tag="mm", bufs=3, name="psp")
                for ki in range(KT):
                    nc.tensor.matmul(
                        ps, wtiles[ki][:, mo * 128:(mo + 1) * 128],
                        rhs_tiles[ki], start=(ki == 0), stop=(ki == KT - 1))
                s = act.tile([128, WD], BF16, tag=tag, bufs=4, name="proj")
                nc.scalar.activation(s, ps, ACTF.Identity, bias=bias_tiles[mo])
                res.append(s)
            return res

        def softmax_pv2(h, e_tiles, va2, ctxTn):
            """PV for BOTH batch halves of one head into one [128, WD] psum;
            lhsT = [V_h | ones64] contiguous group so psum rows 64:128 hold
            the denominator rows; one reciprocal + one multiply-evict."""
            ps = pcx.tile([128, WD], F32, tag="ctx", bufs=2, name="ctxps")
            nkt = len(e_tiles)
            for bh in range(2):
                hsl = slice(bh * T, (bh + 1) * T)
                for kt in range(nkt):
                    nc.tensor.matmul(ps[:, hsl],
                                     va2[bh][kt][:, h * 128:(h + 1) * 128],
                                     e_tiles[kt][:, hsl], start=(kt == 0),
                                     stop=(kt == nkt - 1))
            recD = act.tile([64, WD], F32, tag="recD", bufs=2, name="recD")
            nc.vector.reciprocal(out=recD, in_=ps[64:128, :])
            pi = (h % 2) * 64
            nc.vector.tensor_tensor(out=ctxTn[h // 2][pi:pi + 64, :],
                                    in0=ps[0:64, :], in1=recD, op=ALU.mult)

        def bcast_row(row_ap):
            t = pcx.tile([128, WD], F32, tag="ctx", bufs=2, name="bcast")
            nc.tensor.matmul(t, onr32, row_ap, start=True, stop=True)
            return t

        def layer_norm_T(s_tiles, gb, dests=None):
            """LN over the partition (H) axis of transposed [H, WD] pair."""
            psmu = psc.tile([128, WD], F32, tag="sc", bufs=3, name="psmu")
            for k in range(KT):
                nc.tensor.matmul(psmu[0:1, :], ones128, s_tiles[k],
                                 start=(k == 0), stop=(k == KT - 1))
            pss2 = psc.tile([128, WD], F32, tag="sc", bufs=3, name="pss2")
            for k in range(KT):
                sq = act.tile([128, WD], F32R, tag="sq", bufs=2, name="sq")
                nc.scalar.activation(sq, s_tiles[k], ACTF.Square)
                nc.tensor.matmul(pss2[0:1, :], ones128, sq,
                                 start=(k == 0), stop=(k == KT - 1))

            def stat():
                return act.tile([1, WD], F32R, tag="lnstat", bufs=5,
                                name="lnstat")
            mu, ex2, var, rstd, nm = (stat() for _ in range(5))
            nc.scalar.activation(mu, psmu[0:1, :], ACTF.Copy, scale=1.0 / H)
            nc.scalar.activation(ex2, pss2[0:1, :], ACTF.Copy, scale=1.0 / H)
            nc.vector.scalar_tensor_tensor(var, mu, -1.0, mu,
                                           op0=ALU.mult, op1=ALU.mult)
            nc.vector.tensor_tensor(out=var, in0=ex2, in1=var, op=ALU.add)
            nc.scalar.activation(var, var, ACTF.Sqrt, bias=eps_t)
            with nc.allow_low_precision("f32r rows feed broadcast matmuls"):
                nc.vector.reciprocal(rstd, var)
            nc.vector.scalar_tensor_tensor(nm, mu, -1.0, rstd,
                                           op0=ALU.mult, op1=ALU.mult)
            RS = bcast_row(rstd)
            NM = bcast_row(nm)
            res = []
            for k in range(KT):
                o = (dests[k] if dests is not None else
                     act.tile([128, WD], F32R, tag="lno", bufs=4, name="lno"))
                nc.vector.tensor_tensor(out=o, in0=s_tiles[k], in1=RS,
                                        op=ALU.mult)
                nc.vector.tensor_tensor(out=o, in0=o, in1=NM, op=ALU.add)
                if gb is not None:
                    nc.vector.tensor_scalar(o, o, gb[0][k], gb[1][k],
                                            op0=ALU.mult, op1=ALU.add)
                res.append(o)
            return res

        # ---------------- Stage T: tendency attention + LN1 ----------------
        for p in range(PR):
            if p == 0:
                xTb = xTb0
            else:
                xTb = []
                for k in range(KT):
                    t = act.tile([128, WD], F32R, tag="xT", bufs=4, name="xTt")
                    nc.sync.dma_start(out=t, in_=xT[p, k * 128:(k + 1) * 128, :])
                    xTb.append(t)
            qT = proj_T(wq_t, xTb, tbq_t, "qT")
            ctxTn = [act.tile([128, WD], BF16, tag="ctxTn", bufs=4,
                              name="ctxTn") for _ in range(KT)]
            for h in range(NH):
                pi = (h % 2) * 64
                sps = psc.tile([128, WD], F32, tag="sc", bufs=3, name="scte")
                nc.tensor.matmul(sps[0:E, :], kTe[h // 2][pi:pi + 64, :],
                                 qT[h // 2][pi:pi + 64, :],
                                 start=True, stop=True)
                et = act.tile([E, WD], F32R, tag="et", bufs=3, name="ett")
                nc.scalar.activation(et, sps[0:E, :], ACTF.Exp)
                softmax_pv2(h, [et[0:E, :]], [[vea], [vea]], ctxTn)
            s1 = []
            for mo in range(KT):
                ps = pmm.tile([128, WD], F32, tag="mm", bufs=3, name="psh")
                for ki in range(KT):
                    nc.tensor.matmul(ps, wo_t[ki][:, mo * 128:(mo + 1) * 128],
                                     ctxTn[ki], start=(ki == 0),
                                     stop=(ki == KT - 1))
                s = act.tile([128, WD], F32R, tag="s1", bufs=4, name="s1")
                nc.vector.scalar_tensor_tensor(s, ps, tbo_t[mo], xTb[mo],
                                               op0=ALU.add, op1=ALU.add)
                s1.append(s)
            layer_norm_T(s1, (g1_t, b1_t) if apply_g1b1 else None,
                         dests=htT[p])

        # ---------------- Branch stages ----------------
        for i in range(4):
            wq_b = [wts.tile([128, H], BF16, tag="wh", bufs=32, name="bwq")
                    for _ in range(KT)]
            wk_b = [wts.tile([128, H], BF16, tag="wh", bufs=32, name="bwk")
                    for _ in range(KT)]
            wv_b = [wts.tile([128, H], BF16, tag="wh", bufs=32, name="bwv")
                    for _ in range(KT)]
            wh_b = [wts.tile([128, H], BF16, tag="wh", bufs=32, name="bwh")
                    for _ in range(KT)]
            for k in range(KT):
                sl = slice(k * 128, (k + 1) * 128)
                nc.sync.dma_start(out=wq_b[k], in_=bWq[i, sl, :])
                nc.sync.dma_start(out=wk_b[k], in_=bWk[i, sl, :])
                nc.sync.dma_start(out=wv_b[k], in_=bWv[i, sl, :])
                nc.sync.dma_start(out=wh_b[k], in_=What[i, sl, :])
            gsl = {0: slice(0, 2), 1: slice(5, 9),
                   2: slice(1, 3), 3: slice(3, 5)}[i]
            ng = gsl.stop - gsl.start
            for p in range(PR):
                smp = act.tile([3, 4, WD], F32R, tag="sm", bufs=2, name="sm",
                               padded_shape=None)
                smp = smp[:, 0:ng, :]
                nc.sync.dma_start(out=smp, in_=smalls[p][:, gsl, :])
                sms = [smp[:, :, 0:T], smp[:, :, T:WD]]
                ml = None
                if i == 1:  # local: multiplicative mask band*outer(col,row)+B
                    ml = []
                    for m in range(MT):
                        msl = slice(m * 128, (m + 1) * 128)
                        psA = psc.tile([128, WD], F32, tag="sc", bufs=3,
                                       name="psA")
                        psB = psc.tile([128, WD], F32, tag="sc", bufs=3,
                                       name="psB")
                        for bh in range(2):
                            hsl = slice(bh * T, (bh + 1) * T)
                            nc.tensor.matmul(psA[:, hsl], sms[bh][0:1, 0, msl],
                                             sms[bh][0:1, 1, :],
                                             start=True, stop=True)
                            nc.tensor.matmul(psB[:, hsl], sms[bh][0:1, 3, msl],
                                             sms[bh][0:1, 2, :],
                                             start=True, stop=True)
                        mt_ = act.tile([128, WD], F32, tag="ml", bufs=2,
                                       name="ml")
                        nc.vector.tensor_tensor(out=mt_, in0=psA,
                                                in1=bandt[m], op=ALU.mult)
                        nc.vector.tensor_tensor(out=mt_, in0=mt_, in1=psB,
                                                op=ALU.add)
                        ml.append(mt_)
                qT = proj_T(wq_b, htT[p], bbq_t[i], "qT")
                kT = proj_T(wk_b, htT[p], bbk_t[i], "kT")
                va = [[None] * MT for _ in range(2)]
                for bh in range(2):
                    for mo in range(MT):
                        ps = pmm.tile([128, H], F32, tag="mm", bufs=3,
                                      name="psv")
                        for ki in range(KT):
                            off = bh * T + mo * 128
                            nc.tensor.matmul(
                                ps, htT[p][ki][:, off:off + 128],
                                wv_b[ki], start=(ki == 0), stop=(ki == KT - 1))
                        vt = act.tile([128, VW], F32R, tag="va", bufs=4,
                                      name="va")
                        vt3 = vt.rearrange("p (h d) -> p h d", h=NH)
                        ps3 = ps.rearrange("p (h d) -> p h d", h=NH)
                        nc.scalar.activation(vt3[:, :, 0:64], ps3, ACTF.Copy)
                        nc.vector.memset(vt3[:, :, 64:128].bitcast(
                            mybir.dt.uint32), 0x3F800000)
                        va[bh][mo] = vt
                ctxTn = [act.tile([128, WD], BF16, tag="ctxTn", bufs=4,
                                  name="ctxTn") for _ in range(KT)]
                for h in range(NH):
                    pi = (h % 2) * 64
                    ets = []
                    for m in range(MT):
                        msl = slice(m * 128, (m + 1) * 128)
                        sps = psc.tile([128, WD], F32, tag="sc", bufs=3,
                                       name="scb")
                        for bh in range(2):
                            hsl = slice(bh * T, (bh + 1) * T)
                            nc.tensor.matmul(
                                sps[:, hsl],
                                kT[h // 2][pi:pi + 64, bh * T + m * 128:
                                           bh * T + m * 128 + 128],
                                qT[h // 2][pi:pi + 64, hsl],
                                start=True, stop=(i == 1))
                            if i == 0:
                                nc.tensor.matmul(sps[:, hsl],
                                                 sms[bh][0:1, 0, msl],
                                                 sms[bh][0:1, 1, :],
                                                 start=False, stop=True)
                            elif i in (2, 3):
                                nc.tensor.matmul(sps[:, hsl],
                                                 sms[bh][:, 1, msl],
                                                 sms[bh][:, 0, :],
                                                 start=False, stop=True)
                        et = act.tile([128, WD], F32R, tag="et", bufs=3,
                                      name="etb")
                        nc.scalar.activation(et, sps, ACTF.Exp)
                        if i == 1:
                            nc.vector.tensor_tensor(out=et, in0=et, in1=ml[m],
                                                    op=ALU.mult)
                        ets.append(et)
                    softmax_pv2(h, ets, va, ctxTn)
                for mo in range(KT):
                    ps = pmm.tile([128, WD], F32, tag="mm", bufs=3,
                                  name="psh2")
                    for ki in range(KT):
                        nc.tensor.matmul(
                            ps, wh_b[ki][:, mo * 128:(mo + 1) * 128],
                            ctxTn[ki], start=(ki == 0), stop=(ki == KT - 1))
                    if i == 0:
                        nc.scalar.copy(out=h2sb[p][mo], in_=ps)
                    elif i < 3:
                        nc.vector.tensor_tensor(out=h2sb[p][mo],
                                                in0=h2sb[p][mo], in1=ps,
                                                op=ALU.add)
                    else:
                        nc.vector.scalar_tensor_tensor(
                            h2sb[p][mo], ps, bhat_t[mo], h2sb[p][mo],
                            op0=ALU.add, op1=ALU.add)

        # ---------------- Final: residual + LN2 + W2 ----------------
        w2_t = [wts.tile([128, H], F32R, tag="w", bufs=8, name="w2p")
                for _ in range(KT)]
        for k in range(KT):
            nc.sync.dma_start(out=w2_t[k], in_=W2p[k * 128:(k + 1) * 128, :])
        for p in range(PR):
            for k in range(KT):
                nc.vector.tensor_tensor(out=h2sb[p][k], in0=h2sb[p][k],
                                        in1=htT[p][k], op=ALU.add)
            n2 = layer_norm_T(h2sb[p], None)
            for bh in range(2):
                for mo in range(MT):
                    ps = pmm.tile([128, H], F32, tag="mm", bufs=3, name="pso")
                    for ki in range(KT):
                        off = bh * T + mo * 128
                        nc.tensor.matmul(ps, n2[ki][:, off:off + 128],
                                         w2_t[ki], start=(ki == 0),
                                         stop=(ki == KT - 1))
                    osb = act.tile([128, H], F32, tag="osb", bufs=1,
                                   name="osb")
                    if apply_c2:
                        nc.vector.tensor_tensor(out=osb, in0=ps, in1=c2n,
                                                op=ALU.add)
                    else:
                        nc.scalar.copy(out=osb, in_=ps)
                    nc.sync.dma_start(
                        out=out[2 * p + bh, mo * 128:(mo + 1) * 128, :],
                        in_=osb)
        pcx.release()
        psc.release()
        pmm.release()
        act.release()
        wts.release()
        per.release()
        cst.release()

    nc.compile()
    return nc


def _host_prep(inputs):
    f32 = np.float32
    g = {}
    x = np.asarray(inputs["x"], f32)
    lengths = np.asarray(inputs["lengths"])
    speakers = np.asarray(inputs["speakers"])
    emo = np.asarray(inputs["emo_table"], f32)

    xTa = np.ascontiguousarray(x.transpose(0, 2, 1))  # [B, H, T]
    xTp = np.ascontiguousarray(
        xTa.reshape(B // 2, 2, H, T).transpose(0, 2, 1, 3).reshape(
            B // 2, H, WD))
    j = np.arange(T)
    row = (j[None, :] < lengths[:, None]).astype(f32)
    col = row
    sp = speakers.astype(f32)
    u1 = row * sp
    u2 = row * (1.0 - sp)
    ones = np.ones_like(row)
    z = np.zeros_like(row)
    sm = np.zeros((B, 3, 9, T), f32)
    sm[:, 0, 0] = NEG * (1.0 - col)                               # 0: FR glob
    sm[:, 0, 1], sm[:, 1, 1], sm[:, 2, 1] = ones, u1, u2          # 1: FL
    sm[:, 0, 2], sm[:, 1, 2], sm[:, 2, 2] = (NEG * ones, -NEG * sp,
                                             -NEG * (1.0 - sp))   # 2: FRintra
    sm[:, 0, 3], sm[:, 1, 3], sm[:, 2, 3] = ones, u1, u2          # 3: FL dup
    sm[:, 0, 4], sm[:, 1, 4], sm[:, 2, 4] = (NEG * ones,
                                             -NEG * (1.0 - sp) * col,
                                             -NEG * sp * col)     # 4: FRinter
    sm[:, 0, 5] = col                                             # 5
    sm[:, 0, 6] = row                                             # 6
    sm[:, 0, 7] = 1.0 - row                                       # 7
    sm[:, 0, 8] = ones[0]                                         # 8

    import ml_dtypes
    band = (np.abs(j[:, None] - j[None, :]) <= 2)
    g["bandd"] = np.concatenate([band, band],
                                axis=1).astype(ml_dtypes.bfloat16)
    kemo = (emo @ np.asarray(inputs["t_Wk"], f32)
            + np.asarray(inputs["t_bk"], f32))
    g["kTemo"] = np.ascontiguousarray(kemo.T).astype(ml_dtypes.bfloat16)
    vemo = (emo @ np.asarray(inputs["t_Wv"], f32)
            + np.asarray(inputs["t_bv"], f32))
    vaug = np.ones((E, VW), f32)
    vaug3 = vaug.reshape(E, NH, 128)
    vaug3[:, :, 0:64] = vemo.reshape(E, NH, 64)
    g["vemoaug"] = vaug
    bf16 = ml_dtypes.bfloat16
    g["tWq"] = np.asarray(inputs["t_Wq"], f32) / np.sqrt(DH).astype(f32)
    g["tWo"] = np.asarray(inputs["t_Wo"], f32).astype(bf16)
    g["bWq"] = (np.asarray(inputs["b_Wq"], f32)
                / np.sqrt(DH).astype(f32)).astype(bf16)
    g["bWk"] = np.asarray(inputs["b_Wk"], f32).astype(bf16)
    g["bWv"] = np.asarray(inputs["b_Wv"], f32).astype(bf16)
    W1 = np.asarray(inputs["W1"], np.float64)
    bWo = np.asarray(inputs["b_Wo"], np.float64)
    g["What"] = np.stack(
        [(bWo[i] @ W1[i * H:(i + 1) * H]).astype(f32) for i in range(4)]
    ).astype(bf16)
    ln2g = np.asarray(inputs["ln2_g"], np.float64)
    g["W2p"] = (ln2g[:, None]
                * np.asarray(inputs["W2"], np.float64)).astype(f32)
    g["onesd"] = np.ones(T, f32)
    bhat = np.asarray(inputs["b1"], np.float64).copy()
    for i in range(4):
        eff = (np.asarray(inputs["b_bo"][i], np.float64)
               + np.asarray(inputs["b_bv"][i], np.float64) @ bWo[i])
        bhat += eff @ W1[i * H:(i + 1) * H]
    vecs = np.zeros((H, 16), f32)
    vecs[:, 0] = np.asarray(inputs["t_bq"], f32) / np.sqrt(DH).astype(f32)
    vecs[:, 1] = (np.asarray(inputs["t_bo"], np.float64)
                  + np.asarray(inputs["t_bv"], np.float64)
                  @ np.asarray(inputs["t_Wo"], np.float64)).astype(f32)
    vecs[:, 2] = bhat.astype(f32)
    vecs[:, 3] = np.asarray(inputs["t_ln_g"], f32)
    vecs[:, 4] = np.asarray(inputs["t_ln_b"], f32)
    vecs[:, 5:9] = (np.asarray(inputs["b_bq"], f32).T
                    / np.sqrt(DH).astype(f32))
    vecs[:, 9:13] = np.asarray(inputs["b_bk"], f32).T
    g["vecs"] = vecs
    g["c2row"] = (np.asarray(inputs["ln2_b"], np.float64)
                  @ np.asarray(inputs["W2"], np.float64)).astype(f32)

    apply_g1b1 = not (np.all(inputs["t_ln_g"] == 1.0)
                      and np.all(inputs["t_ln_b"] == 0.0))
    apply_c2 = bool(np.any(g["c2row"] != 0.0))

    # pair-merged smalls: [B//2, 3, 9, 2T], bh halves side by side
    smP = np.ascontiguousarray(
        sm.reshape(B // 2, 2, 3, 9, T).transpose(0, 2, 3, 1, 4).reshape(
            B // 2, 3, 9, WD))
    in_maps = []
    for c in range(NCORES):
        m = dict(g)
        m["xT"] = np.ascontiguousarray(xTp[c * PR:(c + 1) * PR])
        m["smalls"] = np.ascontiguousarray(smP[c * PR:(c + 1) * PR])
        in_maps.append(m)
    return in_maps, apply_g1b1, apply_c2


def kernel(**inputs):
    in_maps, apply_g1b1, apply_c2 = _host_prep(inputs)
    key = (apply_g1b1, apply_c2)
    if key not in _CACHE:
        _CACHE[key] = _build(*key)
    nc = _CACHE[key]
    res = run_bass_kernel_spmd(nc, in_maps, core_ids=list(range(NCORES)),
                               trace=False)
    outs = [res.results[c]["out"] for c in range(NCORES)]
    return np.concatenate(outs, axis=0)



# revision 48
# speedup vs baseline: 1.2535x; 1.1872x over previous
"""DialogueEIN fused kernel for 8 TRN2 NeuronCores (data-parallel over batch).

Self-contained: hardcodes shapes for the nn_DialogueEIN problem
  x[64,256,512], T=256, H=512, NH=8 heads, E=7 emotion slots, window 5.

Strategy (per core, 8 batches, processed as 4 batch-PAIRS):
  - All activations live in "transposed" space [H, T] so attention scores are
    computed directly as S_T[k, j] (keys on partitions, queries on free dim):
    qT/kT come straight out of the projection matmuls; softmax needs no
    transposes anywhere.  Two batches share each tile on the free dim
    ([128, 512]) so projection/LN/exp instruction counts halve.
  - Softmax without max-subtraction: scores are O(1) and the additive mask
    bias is -50 instead of -1e4 (identical through softmax: fully-masked rows
    reduce to the reference's plain softmax; partially-masked rows leave
    masked weights at ~e^-48 relative -- below fp32 noise).
  - The PV matmul's lhsT is a contiguous [V_h | ones64] 128-column group, so
    PSUM rows 64:128 hold 64 broadcast copies of the softmax denominator row;
    normalization is a 64-lane reciprocal + the eviction multiply.
  - Mask biases for global/intra/inter are rank<=3 outer products accumulated
    into the score PSUM by tiny extra matmuls (host ships factor vectors).
    The local sliding-window branch uses a multiplicative post-exp mask
    built per pair from a constant band matrix (4 rank-1 matmuls + 2 DVE ops).
  - Host folds: b_Wo[i] @ W1_i (kills the concat+W1 matmul), ln2 gamma/beta
    into W2, t_bv/b_bv into downstream biases, 1/sqrt(dh) into Wq.
  - LayerNorm over the partition axis: ones-column matmuls for mean/E[x^2],
    PE rank-1 broadcast of rstd / (-mu*rstd) rows, per-partition gamma/beta.
  - All big matmuls run as float32r (full-rate fp32 mode on the PE).
"""

import numpy as np

import concourse.bass as bass
import concourse.mybir as mybir
import concourse.tile as tile
from concourse import bacc
from concourse.bass_utils import run_bass_kernel_spmd

F32 = mybir.dt.float32
F32R = mybir.dt.float32r
BF16 = mybir.dt.bfloat16
ALU = mybir.AluOpType
ACTF = mybir.ActivationFunctionType

B, T, H, NH, E = 64, 256, 512, 8, 7
DH = H // NH
NCORES = 8
BL = B // NCORES          # 8 batches per core
PR = BL // 2              # 4 batch-pairs per core
WD = 2 * T                # 512: paired free width
VW = NH * 128             # 1024: V_aug width ([V_h | ones64] per head)
NEG = -50.0
KT = H // 128             # 4
MT = T // 128             # 2
EPS = 1e-12

_CACHE = {}


def _build(apply_g1b1, apply_c2):
    nc = bacc.Bacc("TRN2", target_bir_lowering=False, debug=False,
                   enable_asserts=False)

    def din(name, shape, dt=F32R):
        return nc.dram_tensor(name, list(shape), dt, kind="ExternalInput").ap()

    xT = din("xT", (PR, H, WD))
    smalls = din("smalls", (PR, 3, 9, WD))
    # key-validity mask, keys on partitions: [PR, 128, MT*2] col idx = mo*2+bh
    colm = din("colm", (PR, 128, MT * 2), F32)
    bandd = din("bandd", (T, WD), mybir.dt.bfloat16)
    kTemo = din("kTemo", (H, E), BF16)
    vemoaug = din("vemoaug", (E, VW))
    tWq = din("tWq", (H, H))
    tWo = din("tWo", (H, H), BF16)
    bWq = din("bWq", (4, H, H), BF16)
    bWk = din("bWk", (4, H, H), BF16)
    bWv = din("bWv", (4, H, H), BF16)
    What = din("What", (4, H, H), BF16)
    W2p = din("W2p", (H, H))
    onesd = din("onesd", (T,))
    # packed per-partition vectors: [H, 16] cols:
    # 0 tbq, 1 tbo, 2 bhat, 3 g1, 4 b1v, 5:9 bbq[0..3], 9:13 bbk[0..3]
    vecs = din("vecs", (H, 16), F32)
    c2row = din("c2row", (H,), F32)
    out = nc.dram_tensor("out", [BL, T, H], F32, kind="ExternalOutput").ap()

    with tile.TileContext(nc) as tc:
        cst = tc.alloc_tile_pool(name="cst", bufs=1)
        per = tc.alloc_tile_pool(name="per", bufs=1)
        wts = tc.alloc_tile_pool(name="wts", bufs=1)
        act = tc.alloc_tile_pool(name="act", bufs=1)
        pmm = tc.alloc_tile_pool(name="pmm", bufs=3, space="PSUM")
        psc = tc.alloc_tile_pool(name="psc", bufs=3, space="PSUM")
        pcx = tc.alloc_tile_pool(name="pcx", bufs=2, space="PSUM")

        # ---- startup-critical loads first: tendency weights + pair-0 xT
        # interleaved so the first projection matmuls can begin ASAP.
        wq_t = [wts.tile([128, H], F32R, tag="w", bufs=4, name="twq")
                for _ in range(KT)]
        wo_t = [wts.tile([128, H], BF16, tag="wh", bufs=28, name="two")
                for _ in range(KT)]
        xTb0 = [act.tile([128, WD], F32R, tag="xT", bufs=4, name="xTt")
                for _ in range(KT)]
        for k in range(KT):
            nc.sync.dma_start(out=wq_t[k], in_=tWq[k * 128:(k + 1) * 128, :])
            nc.sync.dma_start(out=xTb0[k],
                              in_=xT[0, k * 128:(k + 1) * 128, :])
        vec_t = []
        for k in range(KT):
            t = cst.tile([128, 16], F32, name=f"vecs{k}")
            nc.sync.dma_start(out=t, in_=vecs[k * 128:(k + 1) * 128, :])
            vec_t.append(t)
        ones128 = cst.tile([128, 1], F32R, name="ones128")
        nc.sync.dma_start(out=ones128, in_=onesd[0:128])
        ones128b = cst.tile([128, 1], BF16, name="ones128b")
        nc.gpsimd.memset(ones128b, 1.0)
        onr32 = cst.tile([1, 128], F32R, name="onr32")
        nc.sync.dma_start(out=onr32, in_=onesd[0:128])
        eps_t = cst.tile([1, 1], F32, name="eps_t")
        nc.vector.memset(eps_t, EPS)
        kTe = []
        for k in range(KT):
            t = cst.tile([128, E], BF16, name=f"kTemo{k}")
            nc.sync.dma_start(out=t, in_=kTemo[k * 128:(k + 1) * 128, :])
            kTe.append(t)
        vea = cst.tile([E, VW], F32R, name="vemoaug")
        nc.sync.dma_start(out=vea, in_=vemoaug)
        for k in range(KT):
            nc.sync.dma_start(out=wo_t[k], in_=tWo[k * 128:(k + 1) * 128, :])
        bandt = []
        for m in range(MT):
            t = cst.tile([128, WD], mybir.dt.bfloat16, name=f"band{m}")
            nc.sync.dma_start(out=t, in_=bandd[m * 128:(m + 1) * 128, :])
            bandt.append(t)

        def vcol(j):
            return [vec_t[k][:, j:j + 1] for k in range(KT)]

        tbq_t = vcol(0)
        tbo_t = vcol(1)
        bhat_t = vcol(2)
        g1_t = vcol(3) if apply_g1b1 else None
        b1_t = vcol(4) if apply_g1b1 else None
        bbq_t = [vcol(5 + i) for i in range(4)]
        bbk_t = [vcol(9 + i) for i in range(4)]
        c2n = None
        if apply_c2:
            c2n = cst.tile([128, H], F32, name="c2n")
            nc.sync.dma_start(
                out=c2n, in_=bass.AP(tensor=c2row.tensor, offset=c2row.offset,
                                     ap=[[0, 128], [1, H]]))

        # persistent per-pair state (htT bf16: feeds bf16 matmuls)
        htT = [[per.tile([128, WD], BF16, name=f"htT_{p}_{k}")
                for k in range(KT)] for p in range(PR)]
        h2sb = [[per.tile([128, WD], F32R, name=f"h2sb_{p}_{k}")
                 for k in range(KT)] for p in range(PR)]

        def proj_T(wtiles, rhs_tiles, bias_tiles, tag):
            """[H, WD] = W.T @ rhs(pair), +bias per-partition (ACT evict)."""
            res = []
            for mo in range(KT):
                ps = pmm.tile([128, WD], F32, tag="mm", bufs=3, name="psp")
                for ki in range(KT):
                    nc.tensor.matmul(
                        ps, wtiles[ki][:, mo * 128:(mo + 1) * 128],
                        rhs_tiles[ki], start=(ki == 0), stop=(ki == KT - 1))
                s = act.tile([128, WD], BF16, tag=tag, bufs=4, name="proj")
                nc.scalar.activation(s, ps, ACTF.Identity, bias=bias_tiles[mo])
                res.append(s)
            return res

        def softmax_pv2(h, e_tiles, va2, ctxTn):
            """PV for BOTH batch halves of one head into one [128, WD] psum;
            lhsT = [V_h | ones64] contiguous group so psum rows 64:128 hold
            the denominator rows; one reciprocal + one multiply-evict."""
            ps = pcx.tile([128, WD], F32, tag="ctx", bufs=2, name="ctxps")
            nkt = len(e_tiles)
            for bh in range(2):
                hsl = slice(bh * T, (bh + 1) * T)
                for kt in range(nkt):
                    nc.tensor.matmul(ps[:, hsl],
                                     va2[bh][kt][:, h * 128:(h + 1) * 128],
                                     e_tiles[kt][:, hsl], start=(kt == 0),
                                     stop=(kt == nkt - 1))
            recD = act.tile([64, WD], F32, tag="recD", bufs=2, name="recD")
            nc.vector.reciprocal(out=recD, in_=ps[64:128, :])
            pi = (h % 2) * 64
            nc.vector.tensor_tensor(out=ctxTn[h // 2][pi:pi + 64, :],
                                    in0=ps[0:64, :], in1=recD, op=ALU.mult)

        def bcast_row(row_ap):
            t = pcx.tile([128, WD], F32, tag="ctx", bufs=2, name="bcast")
            nc.tensor.matmul(t, onr32, row_ap, start=True, stop=True)
            return t


        def ln_stats(s_tiles):
            """LN stats over the partition (H) axis: returns (rstd, nm)."""
            on1 = ones128b if s_tiles[0].dtype == BF16 else ones128
            psmu = psc.tile([128, WD], F32, tag="sc", bufs=3, name="psmu")
            for k in range(KT):
                nc.tensor.matmul(psmu[0:1, :], on1, s_tiles[k],
                                 start=(k == 0), stop=(k == KT - 1))
            pss2 = psc.tile([128, WD], F32, tag="sc", bufs=3, name="pss2")
            for k in range(KT):
                sq = act.tile([128, WD], F32R, tag="sq", bufs=2, name="sq")
                nc.scalar.activation(sq, s_tiles[k], ACTF.Square)
                nc.tensor.matmul(pss2[0:1, :], ones128, sq,
                                 start=(k == 0), stop=(k == KT - 1))

            def stat():
                return act.tile([1, WD], F32R, tag="lnstat", bufs=4,
                                name="lnstat")
            mu, ex2, var = (stat() for _ in range(3))
            rstd = act.tile([1, WD], F32R, tag="lnp", bufs=8, name="rstd")
            nm = act.tile([1, WD], F32R, tag="lnp", bufs=8, name="nm")
            nc.scalar.activation(mu, psmu[0:1, :], ACTF.Copy, scale=1.0 / H)
            nc.scalar.activation(ex2, pss2[0:1, :], ACTF.Copy, scale=1.0 / H)
            nc.vector.scalar_tensor_tensor(var, mu, -1.0, mu,
                                           op0=ALU.mult, op1=ALU.mult)
            nc.vector.tensor_tensor(out=var, in0=ex2, in1=var, op=ALU.add)
            nc.scalar.activation(var, var, ACTF.Sqrt, bias=eps_t)
            with nc.allow_low_precision("f32r rows feed broadcast matmuls"):
                nc.vector.reciprocal(rstd, var)
            nc.vector.scalar_tensor_tensor(nm, mu, -1.0, rstd,
                                           op0=ALU.mult, op1=ALU.mult)
            return rstd, nm

        def ln_apply(s_tiles, stats, gb, dests=None):
            rstd, nm = stats
            RS = bcast_row(rstd)
            NM = bcast_row(nm)
            res = []
            for k in range(KT):
                o = (dests[k] if dests is not None else
                     act.tile([128, WD], F32R, tag="lno", bufs=4, name="lno"))
                nc.vector.tensor_tensor(out=o, in0=s_tiles[k], in1=RS,
                                        op=ALU.mult)
                nc.vector.tensor_tensor(out=o, in0=o, in1=NM, op=ALU.add)
                if gb is not None:
                    nc.vector.tensor_scalar(o, o, gb[0][k], gb[1][k],
                                            op0=ALU.mult, op1=ALU.add)
                res.append(o)
            return res

        def layer_norm_T(s_tiles, gb, dests=None):
            return ln_apply(s_tiles, ln_stats(s_tiles), gb, dests)

        # ---------------- Stage T: tendency attention + LN1 ----------------
        for p in range(PR):
            if p == 0:
                xTb = xTb0
            else:
                xTb = []
                for k in range(KT):
                    t = act.tile([128, WD], F32R, tag="xT", bufs=4, name="xTt")
                    nc.sync.dma_start(out=t, in_=xT[p, k * 128:(k + 1) * 128, :])
                    xTb.append(t)
            qT = proj_T(wq_t, xTb, tbq_t, "qT")
            ctxTn = [act.tile([128, WD], BF16, tag="ctxTn", bufs=8,
                              name="ctxTn") for _ in range(KT)]
            for h in range(NH):
                pi = (h % 2) * 64
                sps = psc.tile([128, WD], F32, tag="sc", bufs=3, name="scte")
                nc.tensor.matmul(sps[0:E, :], kTe[h // 2][pi:pi + 64, :],
                                 qT[h // 2][pi:pi + 64, :],
                                 start=True, stop=True)
                et = act.tile([E, WD], F32R, tag="ete", bufs=2, name="ett")
                nc.scalar.activation(et, sps[0:E, :], ACTF.Exp)
                softmax_pv2(h, [et[0:E, :]], [[vea], [vea]], ctxTn)
            s1 = []
            for mo in range(KT):
                ps = pmm.tile([128, WD], F32, tag="mm", bufs=3, name="psh")
                for ki in range(KT):
                    nc.tensor.matmul(ps, wo_t[ki][:, mo * 128:(mo + 1) * 128],
                                     ctxTn[ki], start=(ki == 0),
                                     stop=(ki == KT - 1))
                s = act.tile([128, WD], BF16, tag="s1", bufs=4, name="s1")
                nc.vector.scalar_tensor_tensor(s, ps, tbo_t[mo], xTb[mo],
                                               op0=ALU.add, op1=ALU.add)
                s1.append(s)
            layer_norm_T(s1, (g1_t, b1_t) if apply_g1b1 else None,
                         dests=htT[p])

        # -------- Branch stages: software-pipelined over (i, p) --------
        # PE program order per step n: heads(n) -> front(n+1) -> What(n), so
        # the PE never stalls on DVE's ctx-normalize chain before What.
        def wload(i):
            ws = [[wts.tile([128, H], BF16, tag="wh", bufs=28, name=nm)
                   for _ in range(KT)]
                  for nm in ("bwq", "bwk", "bwv", "bwh")]
            for k in range(KT):
                sl = slice(k * 128, (k + 1) * 128)
                for w4, src in zip(ws, (bWq, bWk, bWv, What)):
                    nc.sync.dma_start(out=w4[k], in_=src[i, sl, :])
            return ws

        def branch_front(i, p, ws):
            wq_b, wk_b, wv_b, _ = ws
            st = {"sms": None, "ml": None, "cm": None}
            if i == 0:
                # global branch: key validity folds into va (eviction scale)
                cm = act.tile([128, MT * 2], F32, tag="cm", bufs=2, name="cm")
                nc.sync.dma_start(out=cm, in_=colm[p])
                st["cm"] = cm
            else:
                gsl = {1: slice(5, 9), 2: slice(1, 3), 3: slice(3, 5)}[i]
                ng = gsl.stop - gsl.start
                smp = act.tile([3, 4, WD], F32R, tag="sm", bufs=2,
                               name="sm", padded_shape=None)
                smp = smp[:, 0:ng, :]
                nc.sync.dma_start(out=smp, in_=smalls[p][:, gsl, :])
                st["sms"] = [smp[:, :, 0:T], smp[:, :, T:WD]]
            if i == 1:  # local: multiplicative mask band*outer(col,row)+B
                sms = st["sms"]
                ml = []
                for m in range(MT):
                    msl = slice(m * 128, (m + 1) * 128)
                    psA = psc.tile([128, WD], F32, tag="sc", bufs=3,
                                   name="psA")
                    psB = psc.tile([128, WD], F32, tag="sc", bufs=3,
                                   name="psB")
                    for bh in range(2):
                        hsl = slice(bh * T, (bh + 1) * T)
                        nc.tensor.matmul(psA[:, hsl], sms[bh][0:1, 0, msl],
                                         sms[bh][0:1, 1, :],
                                         start=True, stop=True)
                        nc.tensor.matmul(psB[:, hsl], sms[bh][0:1, 3, msl],
                                         sms[bh][0:1, 2, :],
                                         start=True, stop=True)
                    mt_ = act.tile([128, WD], BF16, tag="ml", bufs=4,
                                   name="ml")
                    nc.vector.tensor_tensor(out=mt_, in0=psA,
                                            in1=bandt[m], op=ALU.mult)
                    nc.vector.tensor_tensor(out=mt_, in0=mt_, in1=psB,
                                            op=ALU.add)
                    ml.append(mt_)
                st["ml"] = ml
            st["qT"] = proj_T(wq_b, htT[p], bbq_t[i], "qT")
            st["kT"] = proj_T(wk_b, htT[p], bbk_t[i], "kT")
            va = [[None] * MT for _ in range(2)]
            for bh in range(2):
                for mo in range(MT):
                    ps = pmm.tile([128, H], F32, tag="mm", bufs=3,
                                  name="psv")
                    for ki in range(KT):
                        off = bh * T + mo * 128
                        nc.tensor.matmul(
                            ps, htT[p][ki][:, off:off + 128],
                            wv_b[ki], start=(ki == 0), stop=(ki == KT - 1))
                    vt = act.tile([128, VW], BF16, tag="va", bufs=8,
                                  name="va")
                    vt3 = vt.rearrange("p (h d) -> p h d", h=NH)
                    ps3 = ps.rearrange("p (h d) -> p h d", h=NH)
                    ones_sl = vt3[:, :, 64:128]
                    if i == 0:
                        cmc = st["cm"][:, mo * 2 + bh:mo * 2 + bh + 1]
                        nc.scalar.activation(vt3[:, :, 0:64], ps3,
                                             ACTF.Copy, scale=cmc)
                        nc.gpsimd.memset(ones_sl, 1.0)
                        nc.gpsimd.tensor_scalar(
                            out=ones_sl, in0=ones_sl, scalar1=cmc,
                            scalar2=None, op0=ALU.mult)
                    else:
                        nc.scalar.activation(vt3[:, :, 0:64], ps3,
                                             ACTF.Copy)
                        nc.gpsimd.memset(ones_sl, 1.0)
                    va[bh][mo] = vt
            st["va"] = va
            return st

        def branch_heads(i, p, st):
            qT, kT, va = st["qT"], st["kT"], st["va"]
            ctxTn = [act.tile([128, WD], BF16, tag="ctxTn", bufs=8,
                              name="ctxTn") for _ in range(KT)]
            sms, ml = st["sms"], st["ml"]
            for h in range(NH):
                pi = (h % 2) * 64
                ets = []
                for m in range(MT):
                    msl = slice(m * 128, (m + 1) * 128)
                    sps = psc.tile([128, WD], F32, tag="sc", bufs=3,
                                   name="scb")
                    for bh in range(2):
                        hsl = slice(bh * T, (bh + 1) * T)
                        nc.tensor.matmul(
                            sps[:, hsl],
                            kT[h // 2][pi:pi + 64, bh * T + m * 128:
                                       bh * T + m * 128 + 128],
                            qT[h // 2][pi:pi + 64, hsl],
                            start=True, stop=(i <= 1))
                        if i in (2, 3):
                            nc.tensor.matmul(sps[:, hsl],
                                             sms[bh][:, 1, msl],
                                             sms[bh][:, 0, :],
                                             start=False, stop=True)
                    et = act.tile([128, WD], BF16, tag="et", bufs=4,
                                  name="etb")
                    nc.scalar.activation(et, sps, ACTF.Exp)
                    if i == 1:
                        nc.vector.tensor_tensor(out=et, in0=et, in1=ml[m],
                                                op=ALU.mult)
                    ets.append(et)
                softmax_pv2(h, ets, va, ctxTn)
            return ctxTn

        def branch_what(i, p, ctxTn, ws):
            wh_b = ws[3]
            for mo in range(KT):
                ps = pmm.tile([128, WD], F32, tag="mm", bufs=3,
                              name="psh2")
                for ki in range(KT):
                    nc.tensor.matmul(
                        ps, wh_b[ki][:, mo * 128:(mo + 1) * 128],
                        ctxTn[ki], start=(ki == 0), stop=(ki == KT - 1))
                if i == 0:
                    nc.scalar.copy(out=h2sb[p][mo], in_=ps)
                elif i < 3:
                    nc.vector.tensor_tensor(out=h2sb[p][mo],
                                            in0=h2sb[p][mo], in1=ps,
                                            op=ALU.add)
                else:
                    nc.vector.scalar_tensor_tensor(
                        h2sb[p][mo], ps, bhat_t[mo], h2sb[p][mo],
                        op0=ALU.add, op1=ALU.add)

        def ffront(p):
            """Final stage, stats half: residual add + LN2 stats."""
            for k in range(KT):
                nc.gpsimd.tensor_tensor(out=h2sb[p][k], in0=h2sb[p][k],
                                        in1=htT[p][k], op=ALU.add)
            return ln_stats(h2sb[p])

        def fback(p, stats):
            n2 = ln_apply(h2sb[p], stats, None)
            for bh in range(2):
                for mo in range(MT):
                    ps = pmm.tile([128, H], F32, tag="mm", bufs=3, name="pso")
                    for ki in range(KT):
                        off = bh * T + mo * 128
                        nc.tensor.matmul(ps, n2[ki][:, off:off + 128],
                                         w2_t[ki], start=(ki == 0),
                                         stop=(ki == KT - 1))
                    osb = act.tile([128, H], F32, tag="osb", bufs=2,
                                   name="osb")
                    if apply_c2:
                        nc.vector.tensor_tensor(out=osb, in0=ps, in1=c2n,
                                                op=ALU.add)
                    else:
                        nc.scalar.copy(out=osb, in_=ps)
                    nc.sync.dma_start(
                        out=out[2 * p + bh, mo * 128:(mo + 1) * 128, :],
                        in_=osb)

        its = [(i, p) for i in range(4) for p in range(PR)]
        W = [None] * 4
        W[0] = wload(0)
        w2_t = None
        sts = {0: branch_front(0, 0, W[0])}
        fstats = [None] * PR
        for n, (i, p) in enumerate(its):
            ctxTn = branch_heads(i, p, sts.pop(n))
            if n + 1 < len(its):
                i2, p2 = its[n + 1]
                if p2 == 1 and i2 < 3:
                    W[i2 + 1] = wload(i2 + 1)
                if i2 == 3 and p2 == 1:
                    w2_t = [wts.tile([128, H], F32R, tag="w", bufs=4,
                                     name="w2p") for _ in range(KT)]
                    for k in range(KT):
                        nc.sync.dma_start(
                            out=w2_t[k], in_=W2p[k * 128:(k + 1) * 128, :])
                sts[n + 1] = branch_front(i2, p2, W[i2])
            branch_what(i, p, ctxTn, W[i])
            if i == 3:
                fstats[p] = ffront(p)
        for p in range(PR):
            fback(p, fstats[p])
        pcx.release()
        psc.release()
        pmm.release()
        act.release()
        wts.release()
        per.release()
        cst.release()

    nc.compile()
    return nc


def _host_prep(inputs):
    f32 = np.float32
    g = {}
    x = np.asarray(inputs["x"], f32)
    lengths = np.asarray(inputs["lengths"])
    speakers = np.asarray(inputs["speakers"])
    emo = np.asarray(inputs["emo_table"], f32)

    xTa = np.ascontiguousarray(x.transpose(0, 2, 1))  # [B, H, T]
    xTp = np.ascontiguousarray(
        xTa.reshape(B // 2, 2, H, T).transpose(0, 2, 1, 3).reshape(
            B // 2, H, WD))
    j = np.arange(T)
    row = (j[None, :] < lengths[:, None]).astype(f32)
    col = row
    sp = speakers.astype(f32)
    u1 = row * sp
    u2 = row * (1.0 - sp)
    ones = np.ones_like(row)
    z = np.zeros_like(row)
    sm = np.zeros((B, 3, 9, T), f32)
    sm[:, 0, 0] = NEG * (1.0 - col)                               # 0: FR glob
    sm[:, 0, 1], sm[:, 1, 1], sm[:, 2, 1] = ones, u1, u2          # 1: FL
    sm[:, 0, 2], sm[:, 1, 2], sm[:, 2, 2] = (NEG * ones, -NEG * sp,
                                             -NEG * (1.0 - sp))   # 2: FRintra
    sm[:, 0, 3], sm[:, 1, 3], sm[:, 2, 3] = ones, u1, u2          # 3: FL dup
    sm[:, 0, 4], sm[:, 1, 4], sm[:, 2, 4] = (NEG * ones,
                                             -NEG * (1.0 - sp) * col,
                                             -NEG * sp * col)     # 4: FRinter
    sm[:, 0, 5] = col                                             # 5
    sm[:, 0, 6] = row                                             # 6
    sm[:, 0, 7] = 1.0 - row                                       # 7
    sm[:, 0, 8] = ones[0]                                         # 8

    import ml_dtypes
    band = (np.abs(j[:, None] - j[None, :]) <= 2)
    g["bandd"] = np.concatenate([band, band],
                                axis=1).astype(ml_dtypes.bfloat16)
    kemo = (emo @ np.asarray(inputs["t_Wk"], f32)
            + np.asarray(inputs["t_bk"], f32))
    g["kTemo"] = np.ascontiguousarray(kemo.T).astype(ml_dtypes.bfloat16)
    vemo = (emo @ np.asarray(inputs["t_Wv"], f32)
            + np.asarray(inputs["t_bv"], f32))
    vaug = np.ones((E, VW), f32)
    vaug3 = vaug.reshape(E, NH, 128)
    vaug3[:, :, 0:64] = vemo.reshape(E, NH, 64)
    g["vemoaug"] = vaug
    bf16 = ml_dtypes.bfloat16
    g["tWq"] = np.asarray(inputs["t_Wq"], f32) / np.sqrt(DH).astype(f32)
    g["tWo"] = np.asarray(inputs["t_Wo"], f32).astype(bf16)
    g["bWq"] = (np.asarray(inputs["b_Wq"], f32)
                / np.sqrt(DH).astype(f32)).astype(bf16)
    g["bWk"] = np.asarray(inputs["b_Wk"], f32).astype(bf16)
    g["bWv"] = np.asarray(inputs["b_Wv"], f32).astype(bf16)
    W1 = np.asarray(inputs["W1"], np.float64)
    bWo = np.asarray(inputs["b_Wo"], np.float64)
    g["What"] = np.stack(
        [(bWo[i] @ W1[i * H:(i + 1) * H]).astype(f32) for i in range(4)]
    ).astype(bf16)
    ln2g = np.asarray(inputs["ln2_g"], np.float64)
    g["W2p"] = (ln2g[:, None]
                * np.asarray(inputs["W2"], np.float64)).astype(f32)
    g["onesd"] = np.ones(T, f32)
    bhat = np.asarray(inputs["b1"], np.float64).copy()
    for i in range(4):
        eff = (np.asarray(inputs["b_bo"][i], np.float64)
               + np.asarray(inputs["b_bv"][i], np.float64) @ bWo[i])
        bhat += eff @ W1[i * H:(i + 1) * H]
    vecs = np.zeros((H, 16), f32)
    vecs[:, 0] = np.asarray(inputs["t_bq"], f32) / np.sqrt(DH).astype(f32)
    vecs[:, 1] = (np.asarray(inputs["t_bo"], np.float64)
                  + np.asarray(inputs["t_bv"], np.float64)
                  @ np.asarray(inputs["t_Wo"], np.float64)).astype(f32)
    vecs[:, 2] = bhat.astype(f32)
    vecs[:, 3] = np.asarray(inputs["t_ln_g"], f32)
    vecs[:, 4] = np.asarray(inputs["t_ln_b"], f32)
    vecs[:, 5:9] = (np.asarray(inputs["b_bq"], f32).T
                    / np.sqrt(DH).astype(f32))
    vecs[:, 9:13] = np.asarray(inputs["b_bk"], f32).T
    g["vecs"] = vecs
    g["c2row"] = (np.asarray(inputs["ln2_b"], np.float64)
                  @ np.asarray(inputs["W2"], np.float64)).astype(f32)

    apply_g1b1 = not (np.all(inputs["t_ln_g"] == 1.0)
                      and np.all(inputs["t_ln_b"] == 0.0))
    apply_c2 = bool(np.any(g["c2row"] != 0.0))

    # pair-merged smalls: [B//2, 3, 9, 2T], bh halves side by side
    smP = np.ascontiguousarray(
        sm.reshape(B // 2, 2, 3, 9, T).transpose(0, 2, 3, 1, 4).reshape(
            B // 2, 3, 9, WD))
    # key-validity, keys on partitions: [B//2, 128, MT*2], col = mo*2 + bh
    MT = T // 128
    colP = np.ascontiguousarray(
        col.reshape(B // 2, 2, MT, 128).transpose(0, 3, 2, 1).reshape(
            B // 2, 128, MT * 2))
    in_maps = []
    for c in range(NCORES):
        m = dict(g)
        m["xT"] = np.ascontiguousarray(xTp[c * PR:(c + 1) * PR])
        m["smalls"] = np.ascontiguousarray(smP[c * PR:(c + 1) * PR])
        m["colm"] = np.ascontiguousarray(colP[c * PR:(c + 1) * PR])
        in_maps.append(m)
    return in_maps, apply_g1b1, apply_c2


def kernel(**inputs):
    in_maps, apply_g1b1, apply_c2 = _host_prep(inputs)
    key = (apply_g1b1, apply_c2)
    if key not in _CACHE:
        _CACHE[key] = _build(*key)
    nc = _CACHE[key]
    res = run_bass_kernel_spmd(nc, in_maps, core_ids=list(range(NCORES)),
                               trace=False)
    outs = [res.results[c]["out"] for c in range(NCORES)]
    return np.concatenate(outs, axis=0)

